# revision 1
# baseline (speedup 1.0000x reference)
"""Trainium2 Bass kernel for nn_MMGNNLayer (GAT with edge-reweighted mask).

Reference math:
    aw    = sigmoid(mlp(x[row], x[col])) > 0 always, so the edge MLP affects
            the output only through the mask edge_vals > 0
    h     = einsum('nd,hde->hne', x, W)
    e     = leaky_relu(esrc[:, :, None] + edst[None, :], 0.2)
    attn  = softmax(where(adj > 0, e, -9e15), axis=-1)
    out   = (attn @ h) -> [N, H*DH]

Device kernel (SPMD over 8 cores, 512-row slab each); per score tile
[128 cols x 512 rows], 4 heads x 32 column chunks, one of three recipes
computes lr = leaky(esrc[r] + edst[c]) (A: DVE add + DVE leaky;
B: Pool add + DVE leaky; C: ACT fused Lrelu-with-bias), then:
    ACT     exp -> bf16, 4 chunks stacked
    Pool    multiplicative mask (f8: 1 at live edges, 0 elsewhere) -> exact
            zeros for masked entries
    PE      attn matmul bf16, lhsT = [h | ones] accumulates scores and the
            softmax denominator into PSUM [65, 512]
h is computed on device with bf16 matmuls (heads stacked, free=256);
esrc/edst use separate f32 matmuls so the exp argument keeps full
precision (esrc errors cancel in softmax, edst errors do not).
x^T, the packed W layout, and W@a_src/a_dst are prepared on the host;
the mask is densified per core on the host (edge bucketing = sharding).
"""

import numpy as np

N, D, H, DH, P = 4096, 256, 4, 64, 128
NCORES = 8
RS = N // NCORES          # 512 rows (output nodes) per core
KC = N // P               # 32 column chunks
GK = 4                    # chunks stacked per elementwise group
NG = KC // GK             # 8 groups
SJ = RS // P              # 4 slab chunks / output row blocks
ALPHA = 0.2
NEGM = -1024.0
HW = DH + 1               # 65: head cols + ones column
CW = H * HW               # 260: per-chunk h1 width

_cache = {}


def build(loop_k=1, mix=(16, 0, 16), mm_eng="pool", mask_bf16=False, premask=True, gk=GK, wbufs=3, cmask_pool=False):
    import concourse.bacc as bacc
    import concourse.tile as tile
    import concourse.mybir as mybir
    from concourse.masks import make_identity

    f32 = mybir.dt.float32
    f32r = mybir.dt.float32r
    bf16 = mybir.dt.bfloat16
    f8 = mybir.dt.float8e5
    AF = mybir.ActivationFunctionType
    OP = mybir.AluOpType

    nc = bacc.Bacc(trn_type="TRN2", debug=False)

    xt_d = nc.dram_tensor("xt", [2, P, N], f32, kind="ExternalInput")
    xtb_d = nc.dram_tensor("xtb", [2, P, N], bf16, kind="ExternalInput")
    xst_d = nc.dram_tensor("xst", [2, P, RS], f32, kind="ExternalInput")
    wall_d = nc.dram_tensor("wall", [2, P, 2 * P], bf16, kind="ExternalInput")
    wsd_d = nc.dram_tensor("wsd", [2, P, 2 * H], f32, kind="ExternalInput")
    mask_d = nc.dram_tensor("mask8", [P, KC * RS], f8, kind="ExternalInput")
    out_d = nc.dram_tensor("out", [RS, H * DH], f32, kind="ExternalOutput")

    with tile.TileContext(nc) as tc:
        with (
            tc.tile_pool(name="cpool", bufs=1) as cp,
            tc.tile_pool(name="wpool", bufs=wbufs) as wp,
            tc.tile_pool(name="ppool", bufs=2, space="PSUM") as pp,
        ):
            def body():
                idn = cp.tile([P, P], f32, name="idn")
                make_identity(nc, idn[:])
                onesc = cp.tile([1, P], f32, name="onesc")
                nc.vector.memset(onesc[:], 1.0)

                # ------------- loads (spread over engine DMA queues) -------
                wall = cp.tile([P, 2 * 2 * P], bf16, name="wall")
                wsd = cp.tile([P, 2 * 2 * H], f32, name="wsd")
                xst = cp.tile([P, 2 * RS], f32, name="xst")
                for dc in range(2):
                    nc.scalar.dma_start(out=wsd[:, dc * 2 * H:(dc + 1) * 2 * H],
                                        in_=wsd_d[:][dc])
                    nc.scalar.dma_start(out=xst[:, dc * RS:(dc + 1) * RS],
                                        in_=xst_d[:][dc])
                    nc.scalar.dma_start(out=wall[:, dc * 2 * P:(dc + 1) * 2 * P],
                                        in_=wall_d[:][dc])
                mdt = bf16 if mask_bf16 else f8
                mtb = cp.tile([P, KC * RS], mdt, name="mtb")
                for g in range(NG):
                    nc.gpsimd.dma_start(
                        out=mtb[:, g * GK * RS:(g + 1) * GK * RS],
                        in_=mask_d[:][:, g * GK * RS:(g + 1) * GK * RS])
                xT = [cp.tile([P, N], f32, name=f"xT{dc}") for dc in range(2)]
                xTb = [cp.tile([P, N], bf16, name=f"xTb{dc}") for dc in range(2)]
                for dc in range(2):
                    for q in range(4):
                        eng = nc.sync
                        eng.dma_start(
                            out=xT[dc][:, q * (N // 4):(q + 1) * (N // 4)],
                            in_=xt_d[:][dc, :, q * (N // 4):(q + 1) * (N // 4)])
                        eng2 = nc.gpsimd if q % 2 == 0 else nc.sync
                        eng2.dma_start(
                            out=xTb[dc][:, q * (N // 4):(q + 1) * (N // 4)],
                            in_=xtb_d[:][dc, :, q * (N // 4):(q + 1) * (N // 4)])

                # ------- slab esrc -> esrcB[h] [128, 512] broadcast tiles ----
                psE = pp.tile([P, SJ * 2 * H], f32, tag="pse", name="psE")
                for j in range(SJ):
                    for dc in range(2):
                        nc.tensor.matmul(
                            psE[:, j * 2 * H:(j + 1) * 2 * H],
                            lhsT=xst[:, dc * RS + j * P:dc * RS + (j + 1) * P],
                            rhs=wsd[:, dc * 2 * H:(dc + 1) * 2 * H],
                            start=(dc == 0), stop=(dc == 1))
                esl = wp.tile([P, SJ * 2 * H], f32, tag="esl", name="esl")
                nc.vector.tensor_copy(out=esl[:], in_=psE[:])
                esth = [cp.tile([1, RS], f32, name=f"esth{h}") for h in range(H)]
                for j in range(SJ):
                    for h in range(H):
                        pst = pp.tile([1, P], f32, tag="mm", name=f"pst{j}_{h}")
                        nc.tensor.transpose(
                            pst[:], esl[:, j * 2 * H + h:j * 2 * H + h + 1],
                            idn[:])
                        nc.vector.tensor_copy(
                            out=esth[h][:, j * P:(j + 1) * P], in_=pst[:])
                esrcB = []
                for h in range(H):
                    psB = pp.tile([P, RS], f32, tag="psO", name=f"psB{h}")
                    nc.tensor.matmul(psB[:], lhsT=onesc[:], rhs=esth[h][:],
                                     start=True, stop=True)
                    eb = cp.tile([P, RS], f32, name=f"esrcB{h}")
                    nc.vector.tensor_copy(out=eb[:], in_=psB[:])
                    esrcB.append(eb)

                # ------- phase P: h1 (bf16 + ones cols) and ea (f32) ---------
                h1 = cp.tile([P, KC * CW], bf16, name="h1")
                h1v = h1[:].rearrange("p (k h w) -> p k h w", h=H, w=HW)
                nc.vector.memset(h1v[:, :, :, DH], 1.0)
                ea = cp.tile([P, KC * 2 * H], f32, name="ea")
                pse = pp.tile([P, KC * 2 * H], f32, tag="pse", name="pse")
                for k in range(KC):
                    psh = pp.tile([P, 2 * P], f32, tag="mm", name=f"psh{k}")
                    for dc in range(2):
                        nc.tensor.matmul(
                            psh[:],
                            lhsT=xTb[dc][:, k * P:(k + 1) * P],
                            rhs=wall[:, dc * 2 * P:(dc + 1) * 2 * P],
                            start=(dc == 0), stop=(dc == 1))
                        nc.tensor.matmul(
                            pse[:, k * 2 * H:(k + 1) * 2 * H],
                            lhsT=xT[dc][:, k * P:(k + 1) * P],
                            rhs=wsd[:, dc * 2 * H:(dc + 1) * 2 * H],
                            start=(dc == 0), stop=(dc == 1))
                    if k % 16 < 6:
                        nc.scalar.copy(
                            out=h1v[:, k, :, 0:DH],
                            in_=psh[:].rearrange("p (h w) -> p h w", w=DH))
                    else:
                        nc.vector.tensor_copy(
                            out=h1v[:, k, :, 0:DH],
                            in_=psh[:].rearrange("p (h w) -> p h w", w=DH))
                    # per-group ea copies so phase A group g only waits on
                    # its own chunks
                    if k % GK == GK - 1:
                        g0 = (k // GK) * GK * 2 * H
                        nc.vector.tensor_copy(
                            out=ea[:, g0:(k + 1) * 2 * H],
                            in_=pse[:, g0:(k + 1) * 2 * H])

                # ------- phase A: masked softmax attention -------------------
                # Per group of 4 chunks, one of three recipes computes
                # lr = leaky(esrc + edst); then ACT exp (stacked) and a Pool
                # multiplicative mask (0/1 f8) produce the bf16 attn weights.
                #   A: DVE ts_add per chunk + DVE leaky stacked
                #   B: Pool ts_add per chunk + DVE leaky stacked
                #   C: ACT Lrelu(esrcB + edst_bias, alpha) per chunk (fused)
                ng = KC // gk
                tg = H * ng
                nA = round(mix[0] * tg / 32)
                nC = round(mix[2] * tg / 32)
                nB = tg - nA - nC
                cnt = {"A": nA, "B": nB, "C": nC}
                recipes = []
                while len(recipes) < tg:
                    for r in ("C", "A", "B", "A"):
                        if len(recipes) == tg:
                            break
                        if cnt[r] > 0:
                            recipes.append(r)
                            cnt[r] -= 1
                mtb3 = mtb[:].rearrange("p (k f) -> p k f", f=RS)
                outsb = [cp.tile([P, H * DH], f32, name=f"outsb{j}")
                         for j in range(SJ)]
                for h in range(H):
                    psO = pp.tile([HW, RS], f32, tag="psO", name=f"psO{h}")
                    for g in range(ng):
                        rc = recipes[h * ng + g]
                        lr = wp.tile([P, gk * RS], f32, tag="lr")
                        gsl = slice(g * gk * RS, (g + 1) * gk * RS)
                        if premask and rc == "A":
                            # v2-style: additive mask fused into the rank-1
                            # add (mask holds 0 / -1024)
                            sstk = wp.tile([P, gk * RS], f32, tag="s")
                            for j in range(gk):
                                k = g * gk + j
                                nc.vector.scalar_tensor_tensor(
                                    out=sstk[:, j * RS:(j + 1) * RS],
                                    in0=esrcB[h][:],
                                    scalar=ea[:, k * 2 * H + H + h:
                                              k * 2 * H + H + h + 1],
                                    in1=mtb[:, k * RS:(k + 1) * RS],
                                    op0=OP.add, op1=OP.add)
                            nc.vector.scalar_tensor_tensor(
                                out=lr[:], in0=sstk[:], scalar=ALPHA,
                                in1=sstk[:], op0=OP.mult, op1=OP.max)
                        elif rc == "C":
                            for j in range(gk):
                                k = g * gk + j
                                nc.scalar.activation(
                                    out=lr[:, j * RS:(j + 1) * RS],
                                    in_=esrcB[h][:], func=AF.Prelu,
                                    bias=ea[:, k * 2 * H + H + h:
                                            k * 2 * H + H + h + 1],
                                    alpha=ALPHA)
                            if premask:
                                lrm = wp.tile([P, gk * RS], f32, tag="s")
                                cmeng = (nc.gpsimd if cmask_pool else
                                         nc.vector)
                                cmeng.tensor_tensor(
                                    out=lrm[:], in0=lr[:], in1=mtb[:, gsl],
                                    op=OP.add)
                                lr = lrm
                        else:
                            sstk = wp.tile([P, gk * RS], f32, tag="s")
                            aeng = nc.vector if rc == "A" else nc.gpsimd
                            for j in range(gk):
                                k = g * gk + j
                                aeng.tensor_scalar_add(
                                    out=sstk[:, j * RS:(j + 1) * RS],
                                    in0=esrcB[h][:],
                                    scalar1=ea[:, k * 2 * H + H + h:
                                               k * 2 * H + H + h + 1])
                            nc.vector.scalar_tensor_tensor(
                                out=lr[:], in0=sstk[:], scalar=ALPHA,
                                in1=sstk[:], op0=OP.mult, op1=OP.max)
                        pe = wp.tile([P, gk * RS], bf16, tag="pe")
                        nc.scalar.activation(out=pe[:], in_=lr[:], func=AF.Exp)
                        if premask:
                            pt = pe
                        else:
                            pt = wp.tile([P, gk * RS], bf16, tag="pt")
                            meng = nc.gpsimd if mm_eng == "pool" else nc.vector
                            meng.tensor_tensor(
                                out=pt[:], in0=pe[:], in1=mtb[:, gsl],
                                op=OP.mult)
                        for j in range(gk):
                            k = g * gk + j
                            nc.tensor.matmul(
                                psO[:], lhsT=h1v[:, k, h, :],
                                rhs=pt[:, j * RS:(j + 1) * RS],
                                start=(k == 0), stop=(k == KC - 1))
                    sO = wp.tile([HW, RS], f32, tag="sO")
                    nc.scalar.copy(out=sO[:], in_=psO[:])
                    for j in range(SJ):
                        psT2 = pp.tile([P, HW], f32, tag="mm",
                                       name=f"psT{h}_{j}")
                        nc.tensor.transpose(psT2[:], sO[:, j * P:(j + 1) * P],
                                            idn[:HW, :HW])
                        rec = wp.tile([P, 1], f32, tag="rec")
                        nc.vector.reciprocal(out=rec[:], in_=psT2[:, DH:DH + 1])
                        nc.vector.tensor_scalar_mul(
                            out=outsb[j][:, h * DH:(h + 1) * DH],
                            in0=psT2[:, 0:DH], scalar1=rec[:])
                for j in range(SJ):
                    nc.sync.dma_start(out=out_d[:][j * P:(j + 1) * P, :],
                                      in_=outsb[j][:])

            if loop_k > 1:
                with tc.For_i(0, loop_k, 1):
                    body()
            else:
                body()

    nc.compile()
    return nc


def _host_prep(inputs, mask_bf16=False, premask=True):
    """Per-core input maps: xt/xst/wall/wsd shared, mask8 per core.

    Sharding: core c owns output rows [c*512, (c+1)*512); edges are bucketed
    by destination row block into the densified per-core mask.
    """
    x = np.ascontiguousarray(np.asarray(inputs["x"], dtype=np.float32))
    W = np.ascontiguousarray(np.asarray(inputs["W"], dtype=np.float32))
    a_src = np.asarray(inputs["a_src"], dtype=np.float32)
    a_dst = np.asarray(inputs["a_dst"], dtype=np.float32)
    ei = np.asarray(inputs["edge_index"])
    ev = np.asarray(inputs["edge_vals"], dtype=np.float32)
    row = ei[0].astype(np.int64)
    col = ei[1].astype(np.int64)

    import ml_dtypes
    f8 = ml_dtypes.float8_e5m2
    bf16 = ml_dtypes.bfloat16

    xt = np.ascontiguousarray(x.T).reshape(2, P, N)
    xtb = xt.astype(bf16)
    wall = np.empty((2, P, 2 * P), np.float32)
    wsd = np.empty((2, P, 2 * H), np.float32)
    for dc in range(2):
        wall[dc] = W[:, dc * P:(dc + 1) * P, :].transpose(1, 0, 2).reshape(P, 2 * P)
        for h in range(H):
            wsd[dc, :, h] = W[h, dc * P:(dc + 1) * P, :] @ a_src[h]
            wsd[dc, :, H + h] = W[h, dc * P:(dc + 1) * P, :] @ a_dst[h]
    wall = wall.astype(bf16)

    live = ev > 0.0
    in_maps = []
    for c in range(NCORES):
        r0 = c * RS
        sel = (row >= r0) & (row < r0 + RS)
        rsel = row[sel] - r0
        csel = col[sel]
        lsel = live[sel]
        mdt = bf16 if mask_bf16 else f8
        if premask:
            m8 = np.full((N, RS), mdt(np.float32(NEGM)), dtype=mdt)
            m8[csel, rsel] = np.where(lsel, mdt(0.0), mdt(np.float32(NEGM)))
        else:
            m8 = np.zeros((N, RS), dtype=mdt)
            m8[csel, rsel] = np.where(lsel, mdt(1.0), mdt(0.0))
        m8 = m8.reshape(KC, P, RS).transpose(1, 0, 2).reshape(P, KC * RS)
        xst = np.ascontiguousarray(xt[:, :, r0:r0 + RS])
        in_maps.append({
            "xt": xt,
            "xtb": xtb,
            "xst": xst,
            "wall": wall,
            "wsd": wsd,
            "mask8": np.ascontiguousarray(m8),
        })
    return in_maps


def _get_runner(nc):
    """jit-compiled 8-core shard_map runner for a built program, cached."""
    import jax
    import concourse.mybir as mybir
    from jax.experimental.shard_map import shard_map
    from jax.sharding import Mesh, PartitionSpec
    from concourse import bass2jax as B

    B.install_neuronx_cc_hook()
    part_name = nc.partition_id_tensor.name if nc.partition_id_tensor else None
    in_names, out_names, out_avals, zero_outs = [], [], [], []
    for alloc in nc.m.functions[0].allocations:
        if not isinstance(alloc, mybir.MemoryLocationSet):
            continue
        name = alloc.memorylocations[0].name
        if alloc.kind == "ExternalInput":
            if name != part_name:
                in_names.append(name)
        elif alloc.kind == "ExternalOutput":
            out_names.append(name)
            shape = tuple(alloc.tensor_shape)
            dtype = mybir.dt.np(alloc.dtype)
            out_avals.append(jax.core.ShapedArray(shape, dtype))
            zero_outs.append(np.zeros(shape, dtype))
    n_params = len(in_names)
    n_outs = len(out_avals)
    all_names = in_names + out_names
    if part_name is not None:
        all_names = all_names + [part_name]

    def _body(*args):
        operands = list(args)
        if part_name is not None:
            operands.append(B.partition_id_tensor())
        outs = B._bass_exec_p.bind(
            *operands, out_avals=tuple(out_avals), in_names=tuple(all_names),
            out_names=tuple(out_names), lowering_input_output_aliases=(),
            sim_require_finite=True, sim_require_nnan=True, nc=nc)
        return tuple(outs)

    devices = jax.devices()[:NCORES]
    mesh = Mesh(np.asarray(devices), ("core",))
    sharded = jax.jit(
        shard_map(_body, mesh=mesh,
                  in_specs=(PartitionSpec("core"),) * (n_params + n_outs),
                  out_specs=(PartitionSpec("core"),) * n_outs, check_rep=False),
        keep_unused=True)
    shard = jax.sharding.NamedSharding(mesh, PartitionSpec("core"))
    return {
        "fn": sharded, "shard": shard, "in_names": in_names,
        "out_names": out_names, "zero_outs": zero_outs,
    }


def _device_inputs(runner, in_maps):
    """device_put concatenated per-core inputs, cached by content hash."""
    import hashlib
    import jax
    key_h = hashlib.md5()
    concat = []
    for nm in runner["in_names"]:
        a = np.concatenate([np.asarray(in_maps[c][nm]) for c in range(NCORES)], 0)
        key_h.update(nm.encode())
        key_h.update(a.tobytes())
        concat.append(a)
    key = key_h.hexdigest()
    ck = ("dev_in", id(runner["fn"]), key)
    if ck not in _cache:
        _cache[ck] = [jax.device_put(a, runner["shard"]) for a in concat]
    return _cache[ck]


def _run(runner, in_maps):
    import jax
    dev_in = _device_inputs(runner, in_maps)
    zk = ("zeros", id(runner["fn"]))
    if zk not in _cache:
        # the kernel writes every output element, so these buffers only need
        # to exist (not stay zero) and can be reused across calls
        _cache[zk] = [
            jax.device_put(np.concatenate([z] * NCORES, 0), runner["shard"])
            for z in runner["zero_outs"]]
    outs = runner["fn"](*dev_in, *_cache[zk])
    jax.block_until_ready(outs)
    return outs


def kernel(**inputs):
    if "nc" not in _cache:
        _cache["nc"] = build(loop_k=1)
        _cache["runner"] = _get_runner(_cache["nc"])
    runner = _cache["runner"]
    in_maps = _host_prep(inputs)
    outs = _run(runner, in_maps)
    out_full = np.asarray(outs[runner["out_names"].index("out")])
    return out_full.astype(np.float32)



# revision 42
# speedup vs baseline: 1.3295x; 1.3295x over previous
"""Trainium2 Bass kernel for nn_MMGNNLayer (GAT with edge-reweighted mask).

Reference math:
    aw    = sigmoid(mlp(x[row], x[col])) > 0 always, so the edge MLP affects
            the output only through the mask edge_vals > 0
    h     = einsum('nd,hde->hne', x, W)
    e     = leaky_relu(esrc[:, :, None] + edst[None, :], 0.2)
    attn  = softmax(where(adj > 0, e, -9e15), axis=-1)
    out   = (attn @ h) -> [N, H*DH]

Key identity (exactly equal to the reference math): with
    leaky(s) = max(s, 0.2 s),  s_ij = esrc_i + edst_j
    exp(leaky(s)) = max(exp(s), exp(0.2 s))
and dividing the whole row by exp(0.2 esrc_i) (cancels in softmax):
    w_ij = max(A_i * C_j, g_j),  A = exp(0.8 esrc), C = exp(edst),
                                 g = exp(0.2 edst)
so the N x N tile work has NO transcendentals: one tensor_scalar
(mult+max, DVE 4x mode in bf16) and one 0/1-mask multiply. A/C/g are
per-node vectors (tiny ACT ops on f32 matmul outputs, so the exp
arguments keep full precision; A's bf16 rounding is row-constant and
cancels in the softmax).

Device kernel (SPMD over 8 cores, 512-row slab each): score tiles are
[128 cols x 512 rows], 4 heads x 8 groups of 4 column chunks. Three
recipes per group spread the two passes across DVE/ACT/Pool:
    R1: DVE ts(max(A*C, g)) + DVE TT mask-mult (bf16 2x)
    R2: ACT Relu(C*A - g) per chunk + Pool stt (r + g) * mask
    R3: DVE ts + Pool TT mask-mult
PE accumulates scores @ [h | ones] into PSUM [65, 512] per head
(numerator + softmax denominator), then a [1,512] reciprocal on ACT and
per-slab transposes finish the normalization. h is computed on device
with bf16 matmuls; esrc/edst use separate f32 matmuls. The 0/1 mask is
densified per core on the host (edge bucketing = the sharding).
"""

import numpy as np

N, D, H, DH, P = 4096, 256, 4, 64, 128
NCORES = 8
RS = N // NCORES          # 512 rows (output nodes) per core
KC = N // P               # 32 column chunks
GK = 4                    # chunks per group
NG = KC // GK             # 8 groups per head
SJ = RS // P              # 4 slab chunks / output row blocks
HW = DH + 1               # 65: head cols + ones column
CW = H * HW               # 260: per-chunk h1 width

_cache = {}


def build(loop_k=1, n2=0, n3=13, wbufs=8):
    import concourse.bacc as bacc
    import concourse.tile as tile
    import concourse.mybir as mybir
    from concourse.masks import make_identity

    f32 = mybir.dt.float32
    f16 = mybir.dt.float16
    bf16 = mybir.dt.bfloat16
    AF = mybir.ActivationFunctionType
    OP = mybir.AluOpType

    nc = bacc.Bacc(trn_type="TRN2", debug=False)

    # host layouts are packed so every input is one (or two) contiguous
    # DMAs: xt/xtb are chunk-interleaved [P, k, dc, 128], the small params
    # dc-major, so DMA issue cost (~1.3us/instruction/queue) stays low.
    xt_d = nc.dram_tensor("xt", [P, 2 * N], f16, kind="ExternalInput")
    xtb_d = nc.dram_tensor("xtb", [P, 2 * N], bf16, kind="ExternalInput")
    xst_d = nc.dram_tensor("xst", [P, 2 * RS], f16, kind="ExternalInput")
    wall_d = nc.dram_tensor("wall", [P, 2 * 2 * P], bf16, kind="ExternalInput")
    wsd_d = nc.dram_tensor("wsd", [P, 2 * 2 * H], f16, kind="ExternalInput")
    mask_d = nc.dram_tensor("maskb", [P, KC * RS], bf16, kind="ExternalInput")
    selb_d = nc.dram_tensor("selb", [H, H * P], bf16, kind="ExternalInput")
    out_d = nc.dram_tensor("out", [RS, H * DH], f32, kind="ExternalOutput")

    # recipe split: n1 = 32 - n2 - n3 groups on R1 (DVE ts + DVE TT), n2 on
    # R4 (Pool ts + DVE TT), n3 on R3 (DVE ts + Pool TT). Slow Pool TTs
    # never sit at a head's first/last group slot — they'd gate the psO
    # accumulation start/stop on PE.
    recipes = ["R1"] * (H * NG)
    elig = [h * NG + p for h in range(H) for p in range(1, NG - 1)]
    for i in range(n3):
        recipes[elig[(i * len(elig)) // n3]] = "R3"
    left = [m for m in range(H * NG) if recipes[m] == "R1"]
    for i in range(n2):
        recipes[left[(i * len(left)) // n2]] = "R4"

    with tile.TileContext(nc) as tc:
        with (
            tc.tile_pool(name="cpool", bufs=1) as cp,
            tc.tile_pool(name="wpool", bufs=wbufs) as wp,
            tc.tile_pool(name="opool", bufs=2) as op,
            tc.tile_pool(name="ppool", bufs=2, space="PSUM") as pp,
        ):
            def body():
                idn = cp.tile([P, P], f32, name="idn")
                make_identity(nc, idn[:])
                onesb = cp.tile([1, P], bf16, name="onesb")
                nc.vector.memset(onesb[:], 1.0)

                # ------------- loads (HWDGE queues only) -------------------
                # The DMA engine is one shared ~400GB/s resource that runs
                # transfers roughly in kick order, so emit in NEED order:
                # small params, then per-quarter (x chunks, then the mask
                # chunks phase A touches first), alternating queues.
                wall = cp.tile([P, 2 * 2 * P], bf16, name="wall")
                wsd = cp.tile([P, 2 * 2 * H], f16, name="wsd")
                xst = cp.tile([P, 2 * RS], f16, name="xst")
                xT = cp.tile([P, 2 * N], f16, name="xT")
                xTb = cp.tile([P, 2 * N], bf16, name="xTb")
                mtb = cp.tile([P, KC * RS], bf16, name="mtb")
                selb = cp.tile([H, H * P], bf16, name="selb")
                NQ = 2 * N // 4
                loads = [
                    (wsd[:], wsd_d[:]),
                    (xst[:], xst_d[:]),
                    (selb[:], selb_d[:]),
                    (wall[:], wall_d[:]),
                ]
                for q in range(4):
                    loads.append((xTb[:, q * NQ:(q + 1) * NQ],
                                  xtb_d[:][:, q * NQ:(q + 1) * NQ]))
                    loads.append((xT[:, q * NQ:(q + 1) * NQ],
                                  xt_d[:][:, q * NQ:(q + 1) * NQ]))
                    for g in (2 * q, 2 * q + 1):
                        loads.append((mtb[:, g * GK * RS:(g + 1) * GK * RS],
                                      mask_d[:][:, g * GK * RS:(g + 1) * GK * RS]))
                # scalar (ACT) queue only for the three phase-E-critical
                # loads — DMA issue costs ~1.3us each on the issuing SEQ,
                # and ACT has compute to do; SP is otherwise idle.
                for i, (dst, src) in enumerate(loads):
                    eng = nc.scalar if i < 3 else nc.sync
                    eng.dma_start(out=dst, in_=src)

                def xt_sl(k, dc):
                    return xT[:, k * 2 * P + dc * P:k * 2 * P + (dc + 1) * P]

                def xtb_sl(k, dc):
                    return xTb[:, k * 2 * P + dc * P:k * 2 * P + (dc + 1) * P]

                # ------- phase E: slab esrc -> A broadcast tiles ------------
                # wsd as weights: one matmul pair gives esrc/edst for all
                # heads of our 512 rows as [2H, 512] directly.
                psE = pp.tile([2 * H, RS], f32, tag="mm", name="psE")
                for dc in range(2):
                    nc.tensor.matmul(
                        psE[:],
                        lhsT=wsd[:, dc * 2 * H:(dc + 1) * 2 * H],
                        rhs=xst[:, dc * RS:(dc + 1) * RS],
                        start=(dc == 0), stop=(dc == 1))
                AallT = cp.tile([H, RS], bf16, name="AallT")
                nc.scalar.activation(out=AallT[:], in_=psE[0:H, :],
                                     func=AF.Exp, scale=0.8)
                Atile = []
                for h in range(H):
                    # selector matmul broadcasts row h of AallT to all 128
                    # partitions (engines can't read a base partition != 0)
                    psA = pp.tile([P, RS], f32, tag="psO", name=f"psA{h}")
                    nc.tensor.matmul(psA[:],
                                     lhsT=selb[:, h * P:(h + 1) * P],
                                     rhs=AallT[:], start=True, stop=True)
                    at = cp.tile([P, RS], bf16, name=f"Atile{h}")
                    nc.scalar.activation(out=at[:], in_=psA[:], func=AF.Copy)
                    Atile.append(at)

                # ------- phase P: h1 (bf16 + ones cols), edst -> C/g --------
                h1 = cp.tile([P, KC * CW], bf16, name="h1")
                h1v = h1[:].rearrange("p (k h w) -> p k h w", h=H, w=HW)
                nc.vector.memset(h1v[:, :, :, DH], 1.0)
                Ct = cp.tile([P, H * KC], f32, name="Ct")
                gt = cp.tile([P, H * KC], f32, name="gt")
                pse = pp.tile([P, KC * 2 * H], f32, tag="pse", name="pse")
                psev = pse[:].rearrange("p (k e) -> p k e", e=2 * H)
                QK = 8  # chunks per C/g flush
                for k in range(KC):
                    if k % 2 == 0:
                        psh = pp.tile([P, 2 * 2 * P], f32, tag="mm",
                                      name=f"psh{k}")
                    for dc in range(2):
                        nc.tensor.matmul(
                            psh[:, (k % 2) * 2 * P:(k % 2 + 1) * 2 * P],
                            lhsT=xtb_sl(k, dc),
                            rhs=wall[:, dc * 2 * P:(dc + 1) * 2 * P],
                            start=(dc == 0), stop=(dc == 1))
                        nc.tensor.matmul(
                            pse[:, k * 2 * H:(k + 1) * 2 * H],
                            lhsT=xt_sl(k, dc),
                            rhs=wsd[:, dc * 2 * H:(dc + 1) * 2 * H],
                            start=(dc == 0), stop=(dc == 1))
                    if k % 2 == 1:
                        nc.scalar.copy(
                            out=h1v[:, k - 1:k + 1, :, 0:DH],
                            in_=psh[:].rearrange("p (c h w) -> p c h w",
                                                 h=H, w=DH))
                    if k % QK == QK - 1:
                        q0 = k + 1 - QK
                        for h in range(H):
                            src = psev[:, q0:k + 1, H + h]
                            nc.scalar.activation(
                                out=Ct[:, h * KC + q0:h * KC + k + 1],
                                in_=src, func=AF.Exp)
                            nc.scalar.activation(
                                out=gt[:, h * KC + q0:h * KC + k + 1],
                                in_=src, func=AF.Exp, scale=0.2)

                # ------- phase A: masked rank-1 max attention ---------------
                mtb3 = mtb[:]
                outsb = [cp.tile([P, H * DH], f32, name=f"outsb{j}")
                         for j in range(SJ)]
                for h in range(H):
                    psO = pp.tile([HW, RS], f32, tag="psO", name=f"psO{h}")
                    for g in range(NG):
                        rc = recipes[h * NG + g]
                        wm = wp.tile([P, GK * RS], bf16, tag="wm")
                        w = wp.tile([P, GK * RS], bf16, tag="w")
                        aeng = nc.gpsimd if rc == "R4" else nc.vector
                        for j in range(GK):
                            k = g * GK + j
                            col = h * KC + k
                            aeng.tensor_scalar(
                                out=w[:, j * RS:(j + 1) * RS],
                                in0=Atile[h][:],
                                scalar1=Ct[:, col:col + 1],
                                scalar2=gt[:, col:col + 1],
                                op0=OP.mult, op1=OP.max)
                            if rc != "R3":
                                nc.vector.tensor_tensor(
                                    out=wm[:, j * RS:(j + 1) * RS],
                                    in0=w[:, j * RS:(j + 1) * RS],
                                    in1=mtb3[:, k * RS:(k + 1) * RS],
                                    op=OP.mult)
                        if rc == "R3":
                            nc.gpsimd.tensor_tensor(
                                out=wm[:], in0=w[:],
                                in1=mtb3[:, g * GK * RS:(g + 1) * GK * RS],
                                op=OP.mult)
                        for j in range(GK):
                            k = g * GK + j
                            nc.tensor.matmul(
                                psO[:], lhsT=h1v[:, k, h, :],
                                rhs=wm[:, j * RS:(j + 1) * RS],
                                start=(k == 0), stop=(k == KC - 1))
                    # per-head output: copy, reciprocal, transpose, normalize
                    sO = op.tile([HW, RS], f32, tag="sO")
                    nc.scalar.copy(out=sO[:], in_=psO[:])
                    rec = op.tile([1, RS], f32, tag="rec")
                    nc.vector.reciprocal(out=rec[:], in_=sO[DH:HW, :])
                    recT = op.tile([P, SJ], f32, tag="recT")
                    for j in range(SJ):
                        psR = pp.tile([P, 1], f32, tag="mm",
                                      name=f"psR{h}_{j}")
                        nc.tensor.transpose(
                            psR[:], rec[:, j * P:(j + 1) * P], idn[:1, :1])
                        nc.vector.tensor_copy(
                            out=recT[:, j:j + 1], in_=psR[:])
                    for j in range(SJ):
                        psT2 = pp.tile([P, HW], f32, tag="mm",
                                       name=f"psT{h}_{j}")
                        nc.tensor.transpose(psT2[:],
                                            sO[:, j * P:(j + 1) * P],
                                            idn[:HW, :HW])
                        nc.scalar.activation(
                            out=outsb[j][:, h * DH:(h + 1) * DH],
                            in_=psT2[:, 0:DH], func=AF.Copy,
                            scale=recT[:, j:j + 1])
                for j in range(SJ):
                    nc.sync.dma_start(out=out_d[:][j * P:(j + 1) * P, :],
                                      in_=outsb[j][:])

            if loop_k > 1:
                with tc.For_i(0, loop_k, 1):
                    body()
            else:
                body()

    nc.compile()
    return nc


def _host_prep(inputs):
    """Per-core input maps: xt/xst/wall/wsd shared, 0/1 bf16 mask per core.

    Sharding: core c owns output rows [c*512, (c+1)*512); edges are bucketed
    by destination row block into the densified per-core mask.
    """
    x = np.ascontiguousarray(np.asarray(inputs["x"], dtype=np.float32))
    W = np.ascontiguousarray(np.asarray(inputs["W"], dtype=np.float32))
    a_src = np.asarray(inputs["a_src"], dtype=np.float32)
    a_dst = np.asarray(inputs["a_dst"], dtype=np.float32)
    ei = np.asarray(inputs["edge_index"])
    ev = np.asarray(inputs["edge_vals"], dtype=np.float32)
    row = ei[0].astype(np.int64)
    col = ei[1].astype(np.int64)

    import ml_dtypes
    bf16 = ml_dtypes.bfloat16

    xt = np.ascontiguousarray(x.T).reshape(2, P, N)
    # chunk-interleaved packed layout [P, k, dc, 128] -> [P, 2N]
    xt2f = np.ascontiguousarray(
        xt.reshape(2, P, KC, P).transpose(1, 2, 0, 3).reshape(P, 2 * N))
    xt2 = xt2f.astype(np.float16)
    xtb2 = xt2f.astype(bf16)
    wall = np.empty((2, P, 2 * P), np.float32)
    wsd = np.empty((2, P, 2 * H), np.float32)
    for dc in range(2):
        wall[dc] = W[:, dc * P:(dc + 1) * P, :].transpose(1, 0, 2).reshape(P, 2 * P)
        for h in range(H):
            wsd[dc, :, h] = W[h, dc * P:(dc + 1) * P, :] @ a_src[h]
            wsd[dc, :, H + h] = W[h, dc * P:(dc + 1) * P, :] @ a_dst[h]
    # dc-major packed [P, 2*...]
    wall2 = np.ascontiguousarray(
        wall.transpose(1, 0, 2).reshape(P, 2 * 2 * P)).astype(bf16)
    wsd2 = np.ascontiguousarray(
        wsd.transpose(1, 0, 2).reshape(P, 2 * 2 * H)).astype(np.float16)

    live = ev > 0.0
    in_maps = []
    for c in range(NCORES):
        r0 = c * RS
        sel = (row >= r0) & (row < r0 + RS)
        rsel = row[sel] - r0
        csel = col[sel]
        lsel = live[sel]
        m8 = np.zeros((N, RS), dtype=bf16)
        m8[csel, rsel] = np.where(lsel, bf16(1.0), bf16(0.0))
        m8 = m8.reshape(KC, P, RS).transpose(1, 0, 2).reshape(P, KC * RS)
        xst = np.ascontiguousarray(
            xt[:, :, r0:r0 + RS].transpose(1, 0, 2).reshape(P, 2 * RS)
        ).astype(np.float16)
        selb = np.zeros((H, H * P), dtype=bf16)
        for h in range(H):
            selb[h, h * P:(h + 1) * P] = bf16(1.0)
        in_maps.append({
            "xt": xt2,
            "xtb": xtb2,
            "xst": xst,
            "wall": wall2,
            "wsd": wsd2,
            "maskb": np.ascontiguousarray(m8),
            "selb": selb,
        })
    return in_maps


def _get_runner(nc):
    """jit-compiled 8-core shard_map runner for a built program, cached."""
    import jax
    import concourse.mybir as mybir
    from jax.experimental.shard_map import shard_map
    from jax.sharding import Mesh, PartitionSpec
    from concourse import bass2jax as B

    B.install_neuronx_cc_hook()
    part_name = nc.partition_id_tensor.name if nc.partition_id_tensor else None
    in_names, out_names, out_avals, zero_outs = [], [], [], []
    for alloc in nc.m.functions[0].allocations:
        if not isinstance(alloc, mybir.MemoryLocationSet):
            continue
        name = alloc.memorylocations[0].name
        if alloc.kind == "ExternalInput":
            if name != part_name:
                in_names.append(name)
        elif alloc.kind == "ExternalOutput":
            out_names.append(name)
            shape = tuple(alloc.tensor_shape)
            dtype = mybir.dt.np(alloc.dtype)
            out_avals.append(jax.core.ShapedArray(shape, dtype))
            zero_outs.append(np.zeros(shape, dtype))
    n_params = len(in_names)
    n_outs = len(out_avals)
    all_names = in_names + out_names
    if part_name is not None:
        all_names = all_names + [part_name]

    def _body(*args):
        operands = list(args)
        if part_name is not None:
            operands.append(B.partition_id_tensor())
        outs = B._bass_exec_p.bind(
            *operands, out_avals=tuple(out_avals), in_names=tuple(all_names),
            out_names=tuple(out_names), lowering_input_output_aliases=(),
            sim_require_finite=True, sim_require_nnan=True, nc=nc)
        return tuple(outs)

    devices = jax.devices()[:NCORES]
    mesh = Mesh(np.asarray(devices), ("core",))
    sharded = jax.jit(
        shard_map(_body, mesh=mesh,
                  in_specs=(PartitionSpec("core"),) * (n_params + n_outs),
                  out_specs=(PartitionSpec("core"),) * n_outs, check_rep=False),
        keep_unused=True)
    shard = jax.sharding.NamedSharding(mesh, PartitionSpec("core"))
    return {
        "fn": sharded, "shard": shard, "in_names": in_names,
        "out_names": out_names, "zero_outs": zero_outs,
    }


def _device_inputs(runner, in_maps):
    """device_put concatenated per-core inputs, cached by content hash."""
    import hashlib
    import jax
    key_h = hashlib.md5()
    concat = []
    for nm in runner["in_names"]:
        a = np.concatenate([np.asarray(in_maps[c][nm]) for c in range(NCORES)], 0)
        key_h.update(nm.encode())
        key_h.update(a.tobytes())
        concat.append(a)
    key = key_h.hexdigest()
    ck = ("dev_in", id(runner["fn"]), key)
    if ck not in _cache:
        _cache[ck] = [jax.device_put(a, runner["shard"]) for a in concat]
    return _cache[ck]


def _run(runner, in_maps):
    import jax
    dev_in = _device_inputs(runner, in_maps)
    zk = ("zeros", id(runner["fn"]))
    if zk not in _cache:
        # the kernel writes every output element, so these buffers only need
        # to exist (not stay zero) and can be reused across calls
        _cache[zk] = [
            jax.device_put(np.concatenate([z] * NCORES, 0), runner["shard"])
            for z in runner["zero_outs"]]
    outs = runner["fn"](*dev_in, *_cache[zk])
    jax.block_until_ready(outs)
    return outs


def kernel(**inputs):
    if "nc" not in _cache:
        _cache["nc"] = build(loop_k=1)
        _cache["runner"] = _get_runner(_cache["nc"])
    runner = _cache["runner"]
    in_maps = _host_prep(inputs)
    outs = _run(runner, in_maps)
    out_full = np.asarray(outs[runner["out_names"].index("out")])
    return out_full.astype(np.float32)
